# revision 3
# baseline (speedup 1.0000x reference)
"""3-layer GAT (GATConv x3 + log_softmax) on 8 trn2 NeuronCores — v3.

Fixed 128-node windows (49/core).  Edge features are pulled with the
hardware SWDGE dma_gather ucode (int16 indices, two 25k-row banks, trailing
-1 indices trimmed descriptor-free).  Rows are [h | al_src | al_dst | pad]
(640 bf16 for layers 1/2, 128 bf16 for layer 3).  al_dst is expanded
per-edge with one-hot transpose matmuls.  Segment softmax + scatter-add via
one-hot selection matmuls in PSUM.  Window outputs are written with plain
contiguous DMAs; node features exchanged with shared-output AllGather.
Built on Bacc (auto library loads for the mlp ucode + event-sem splitting).
"""
import numpy as np
import ml_dtypes

import concourse.bass as bass
import concourse.bacc as bacc
import concourse.mybir as mybir
import concourse.tile as tile
from concourse.bass_utils import run_bass_kernel_spmd

BF = ml_dtypes.bfloat16
N = 50000
NC = 8
SHARD = N // NC            # 6250
BANK = 25000               # int16 index bank split
H, C = 8, 64
F = H * C                  # 512
C3 = 5
F3 = H * C3                # 40
NW = (SHARD + 127) // 128  # 49 windows/core
NPAD = NW * 128            # 6272
KL = KH = 6                # edge tiles per window per bank (recomputed in host_prep)
NEG_SLOPE = 0.2
DT_BF = mybir.dt.bfloat16
DT_F32 = mybir.dt.float32
DT_I16 = mybir.dt.int16
AF = mybir.ActivationFunctionType
ALU = mybir.AluOpType

RW1 = 640                  # [h(512) | als(8) | ald(8) | pad(112)]
RW3 = 56                   # [h3(40) | als3(8) | ald3(8)]
SW1 = F + 16               # staged/written row prefix (528)
SW3 = F3 + 16              # 56


def _split_drain_waits(nc, max_waits=1):
    # kept for the generic null-kernel builder in test harnesses (plain Bass)
    ctr = 0
    for f in nc.m.functions:
        for blk in f.blocks:
            new_list = []
            for ins in blk.instructions:
                if ins.sync_info and \
                        len(ins.sync_info.on_wait) > max_waits:
                    waits = list(ins.sync_info.on_wait)
                    keep, extra = waits[:max_waits], waits[max_waits:]
                    for w in extra:
                        ctr += 1
                        new_list.append(mybir.InstNoOp(
                            name=f"drainfix-{ctr}", engine=ins.engine,
                            ins=[], outs=[],
                            sync_info=mybir.SyncInfo(on_wait=[w], on_update=[])))
                    ins.sync_info.on_wait = keep
                new_list.append(ins)
            blk.instructions[:] = new_list


def _bcast(ap, ap_list):
    """Build an AP over ap's tensor with explicit [step, count] dims."""
    return bass.AP(ap.tensor, ap.offset, ap_list)


def _pack16(flat):
    """[n] int16 -> [128, n/16] wrapped in 16 partitions, replicated x8."""
    return np.tile(flat.reshape(-1, 16).T, (8, 1))


def host_prep(edge_index):
    """Assign edges to dst-owner cores, sort by dst then src-bank, pack
    fixed 128-node windows into (KL | KH) tiles of 128; build tables."""
    global KL, KH
    src = np.concatenate([edge_index[0], np.arange(N, dtype=np.int32)])
    dst = np.concatenate([edge_index[1], np.arange(N, dtype=np.int32)])
    order = np.argsort(dst, kind="stable")
    src, dst = src[order], dst[order]
    per_core = []
    KL = KH = 1
    for c in range(NC):
        lo, hi = c * SHARD, (c + 1) * SHARD
        m0 = np.searchsorted(dst, lo, "left")
        m1 = np.searchsorted(dst, hi, "left")
        s_c, d_c = src[m0:m1], dst[m0:m1]
        w_of = (d_c - lo) // 128
        b = s_c >= BANK
        KL = max(KL, int((np.bincount(w_of[~b], minlength=NW).max() + 127) // 128))
        KH = max(KH, int((np.bincount(w_of[b], minlength=NW).max() + 127) // 128))
        per_core.append((s_c, d_c, w_of, b))
    KT = KL + KH
    ixl = np.zeros((NC, NW, KL * 128), np.int16)
    ixh = np.zeros((NC, NW, KH * 128), np.int16)
    drow = np.full((NC, NW * KT, 128), 999.0, np.float32)
    for c, (s_c, d_c, w_of, b) in enumerate(per_core):
        lo = c * SHARD
        for w in range(NW):
            m = w_of == w
            sl, dl = s_c[m & ~b], d_c[m & ~b] - lo - 128 * w
            sh, dh = s_c[m & b], d_c[m & b] - lo - 128 * w
            ixl[c, w, :len(sl)] = sl
            ixh[c, w, :len(sh)] = sh - BANK
            fr = drow[c, w * KT:(w + 1) * KT].reshape(-1)
            fr[:len(sl)] = dl
            fr[KL * 128:KL * 128 + len(sh)] = dh
    # idx tables in SWDGE wrap layout, one [128, K*8] block per window
    ixl_t = np.zeros((NC, 128, NW * KL * 8), np.int16)
    ixh_t = np.zeros((NC, 128, NW * KH * 8), np.int16)
    for c in range(NC):
        for w in range(NW):
            ixl_t[c, :, w * KL * 8:(w + 1) * KL * 8] = _pack16(ixl[c, w])
            ixh_t[c, :, w * KH * 8:(w + 1) * KH * 8] = _pack16(ixh[c, w])
    # unbanked per-tile tables for the layer-3 indirect gathers
    KT3 = 1
    for (s_c, d_c, w_of, b) in per_core:
        cnt = np.bincount(w_of, minlength=NW)
        KT3 = max(KT3, int((cnt.max() + 127) // 128))
    esrc3 = np.zeros((NC, NW * KT3, 128), np.int32)
    drow3 = np.full((NC, NW * KT3, 128), 999.0, np.float32)
    for c, (s_c, d_c, w_of, b) in enumerate(per_core):
        lo = c * SHARD
        for w in range(NW):
            m = w_of == w
            sw, dw = s_c[m], d_c[m] - lo - 128 * w
            fs = esrc3[c, w * KT3:(w + 1) * KT3].reshape(-1)
            fr = drow3[c, w * KT3:(w + 1) * KT3].reshape(-1)
            fs[:len(sw)] = sw
            fr[:len(sw)] = dw
    return (ixl_t, ixh_t,
            np.ascontiguousarray(drow.transpose(0, 2, 1)).astype(BF), KL, KH,
            np.ascontiguousarray(esrc3.transpose(0, 2, 1)),
            np.ascontiguousarray(drow3.transpose(0, 2, 1)).astype(BF), KT3)


def blockdiag(a):
    Hh, cc = a.shape
    out = np.zeros((Hh * cc, Hh), np.float32)
    for h in range(Hh):
        out[h * cc:(h + 1) * cc, h] = a[h]
    return out


def chunk_rows(m, p=128):
    R, Cc = m.shape
    n = (R + p - 1) // p
    out = np.zeros((n, p, Cc), m.dtype)
    for i in range(n):
        out[i, :min(p, R - i * p)] = m[i * p:(i + 1) * p]
    return out


def build_program(KL, KH, KT3):
    KT = KL + KH
    nc = bacc.Bacc("TRN2", dynamic_dma_scratch_size=65536, num_swdge_queues=4)
    P = {}
    def par(name, shape, dt):
        P[name] = nc.declare_dram_parameter(name, list(shape), dt, isOutput=False)
        return P[name]

    par("xTc", [12, NPAD], DT_F32)
    par("identt", [128, 128], DT_BF)
    par("iotat", [128, 128], DT_BF)
    par("w1", [12, F], DT_BF)
    par("wwa1", [12, 16], DT_BF)
    par("w2c", [4, 128, F], DT_BF)
    par("wwa2c", [4, 128, 16], DT_BF)
    par("w3c", [4, 128, F3], DT_BF)
    par("wwa3c", [4, 128, 16], DT_BF)
    par("b1t", [128, F], DT_BF)
    par("b2t", [128, F], DT_BF)
    par("b3t", [128, C3], DT_F32)
    par("ixl", [128, NW * KL * 8], DT_I16)
    par("ixh", [128, NW * KH * 8], DT_I16)
    par("drow", [128, NW * KT], DT_BF)
    par("esrc3", [128, NW * KT3], mybir.dt.int32)
    par("drow3", [128, NW * KT3], DT_BF)
    OUT = nc.declare_dram_parameter("out", [SHARD, C3], DT_F32, isOutput=True)
    par("tick", [128, 1], DT_F32)
    TOCK = nc.declare_dram_parameter("tock", [128, 1], DT_F32, isOutput=True)

    with tile.TileContext(nc) as tc:
        with (
            tc.tile_pool(name="const", bufs=1) as cp,
            tc.tile_pool(name="sbuf", bufs=3) as sb,
            tc.tile_pool(name="stage", bufs=3) as stg,
            tc.tile_pool(name="psbig", bufs=3, space="PSUM") as p_big,
            tc.tile_pool(name="pssm", bufs=2, space="PSUM") as p_sm,
            tc.tile_pool(name="pstr", bufs=2, space="PSUM") as p_tr,
            tc.tile_pool(name="dram", bufs=1, space="DRAM") as dr,
        ):
            # ---------------- constants / weights ----------------
            ident_bf = cp.tile([128, 128], DT_BF)
            nc.sync.dma_start(out=ident_bf[:], in_=P["identt"][:])
            iota_bf = cp.tile([128, 128], DT_BF)
            nc.sync.dma_start(out=iota_bf[:], in_=P["iotat"][:])

            t_ixl = cp.tile([128, NW * KL * 8], DT_I16)
            nc.sync.dma_start(out=t_ixl[:], in_=P["ixl"][:])
            t_ixh = cp.tile([128, NW * KH * 8], DT_I16)
            nc.sync.dma_start(out=t_ixh[:], in_=P["ixh"][:])
            t_drow = cp.tile([128, NW * KT], DT_BF)
            nc.sync.dma_start(out=t_drow[:], in_=P["drow"][:])
            t_esrc3 = cp.tile([128, NW * KT3], mybir.dt.int32)
            nc.sync.dma_start(out=t_esrc3[:], in_=P["esrc3"][:])
            t_drow3 = cp.tile([128, NW * KT3], DT_BF)
            nc.sync.dma_start(out=t_drow3[:], in_=P["drow3"][:])
            w1 = cp.tile([12, F], DT_BF)
            nc.sync.dma_start(out=w1[:], in_=P["w1"][:])
            wwa1 = cp.tile([12, 16], DT_BF)
            nc.sync.dma_start(out=wwa1[:], in_=P["wwa1"][:])
            w2 = cp.tile([128, 4, F], DT_BF)
            wwa2 = cp.tile([128, 4, 16], DT_BF)
            w3 = cp.tile([128, 4, F3], DT_BF)
            wwa3 = cp.tile([128, 4, 16], DT_BF)
            for ch in range(4):
                nc.sync.dma_start(out=w2[:, ch, :], in_=P["w2c"][ch])
                nc.sync.dma_start(out=wwa2[:, ch, :], in_=P["wwa2c"][ch])
                nc.sync.dma_start(out=w3[:, ch, :], in_=P["w3c"][ch])
                nc.sync.dma_start(out=wwa3[:, ch, :], in_=P["wwa3c"][ch])
            t_b1 = cp.tile([128, F], DT_BF)
            nc.sync.dma_start(out=t_b1[:], in_=P["b1t"][:])
            t_b2 = cp.tile([128, F], DT_BF)
            nc.sync.dma_start(out=t_b2[:], in_=P["b2t"][:])
            t_b3 = cp.tile([128, C3], DT_F32)
            nc.sync.dma_start(out=t_b3[:], in_=P["b3t"][:])
            xTc = cp.tile([12, NPAD], DT_BF)
            nc.gpsimd.dma_start(out=xTc[:], in_=P["xTc"][:])

            # ---------------- DRAM internals ----------------
            exch1 = dr.tile([NPAD, RW1], DT_BF)
            exch2 = dr.tile([NPAD, RW1], DT_BF)
            exch3 = dr.tile([NPAD, RW3], DT_BF)
            HF1 = dr.tile([N, RW1], DT_BF, addr_space="Shared")
            HF2 = dr.tile([N, RW1], DT_BF, addr_space="Shared")
            HF3 = dr.tile([N, RW3], DT_BF, addr_space="Shared")

            def stage_write(ph, pa, FN, exch, w):
                """psum [128,FN] + [128,16] -> bf16 stage -> contiguous DMA."""
                hst = sb.tile([128, FN + 16], DT_BF, tag="hst")
                nc.scalar.activation(hst[:, 0:FN], ph[:], AF.Copy)
                nc.scalar.activation(hst[:, FN:FN + 16], pa[:], AF.Copy)
                nc.sync.dma_start(out=exch[w * 128:(w + 1) * 128, 0:FN + 16],
                                  in_=hst[:])

            # ---------------- layer-1 node phase (sharded) ----------------
            for w in range(NW):
                lhs = xTc[:, w * 128:(w + 1) * 128]
                ph = p_big.tile([128, F], DT_F32, space="PSUM", tag="pbig")
                nc.tensor.matmul(ph[:], lhsT=lhs, rhs=w1[:], start=True, stop=True)
                pa = p_sm.tile([128, 16], DT_F32, space="PSUM", tag="psm")
                nc.tensor.matmul(pa[:], lhsT=lhs, rhs=wwa1[:], start=True, stop=True)
                stage_write(ph, pa, F, exch1, w)

            rg = [list(range(NC))]
            nc.gpsimd.collective_compute("AllGather", ALU.bypass, replica_groups=rg,
                                         ins=[exch1[0:SHARD, :].opt()],
                                         outs=[HF1[:].opt()])

            # ---------------- edge phase (all 3 layers) ----------------
            def edge_phase(layer, Hsrc, exch_l):
                lay3 = layer == 3
                FH = F3 if lay3 else F           # feature width
                CW = C3 if lay3 else C           # per-head channels
                RWG = (F3 + 8) if lay3 else RW1  # gathered row width
                AO = FH                          # als offset in row
                KTl = KT3 if lay3 else KT
                t_dr = t_drow3 if lay3 else t_drow
                for w in range(NW):
                    tb = w * KTl
                    hg = stg.tile([128, KTl, RWG], DT_BF, tag="hg")
                    if lay3:
                        for k in range(KTl):
                            nc.gpsimd.indirect_dma_start(
                                out=hg[:, k, :], out_offset=None, in_=Hsrc[:],
                                in_offset=bass.IndirectOffsetOnAxis(
                                    ap=t_esrc3[:, tb + k:tb + k + 1], axis=0))
                    else:
                        nc.gpsimd.dma_gather(
                            hg[:, 0:KL, :], Hsrc[:],
                            t_ixl[:, w * KL * 8:(w + 1) * KL * 8],
                            KL * 128, KL * 128, RWG, queue_num=(w % 2) * 2)
                        nc.gpsimd.dma_gather(
                            hg[:, KL:KT, :], Hsrc[BANK:, :],
                            t_ixh[:, w * KH * 8:(w + 1) * KH * 8],
                            KH * 128, KH * 128, RWG, queue_num=(w % 2) * 2 + 1)
                    # per-window al_dst rows (own dst nodes, contiguous)
                    ald_w = sb.tile([128, 8], DT_BF, tag="aldw")
                    nc.scalar.dma_start(
                        out=ald_w[:],
                        in_=exch_l[w * 128:(w + 1) * 128, AO + 8:AO + 16])
                    # one-hot dst-selection matrices for the whole window
                    sel = sb.tile([128, KTl, 128], DT_BF, tag="sel")
                    drow_b = _bcast(t_dr[:, tb:tb + KTl],
                                    [t_dr[:].ap[0], [1, KTl], [0, 128]])
                    iota_b = _bcast(iota_bf[:],
                                    [iota_bf[:].ap[0], [0, KTl], [1, 128]])
                    nc.vector.tensor_tensor(out=sel[:], in0=drow_b, in1=iota_b,
                                            op=ALU.is_equal)
                    # transposes: selT[k] = sel[k].T  (psum chunks of <=8)
                    selT = sb.tile([128, KTl, 128], DT_BF, tag="selT")
                    for c0 in range(0, KTl, 8):
                        cn = min(8, KTl - c0)
                        pt = p_tr.tile([128, 8, 128], DT_BF, space="PSUM", tag="ptr")
                        for j in range(cn):
                            nc.tensor.transpose(pt[:, j, :], sel[:, c0 + j, :],
                                                ident_bf[:])
                        if c0 == 0:
                            nc.scalar.activation(selT[:, c0:c0 + cn, :],
                                                 pt[:, 0:cn, :], AF.Copy)
                        else:
                            nc.vector.tensor_copy(out=selT[:, c0:c0 + cn, :],
                                                  in_=pt[:, 0:cn, :])
                    # expand al_dst to edges:  pad[k] = selT[k]^T @ ald_w
                    pad_ps = p_sm.tile([128, KTl, 8], DT_F32, space="PSUM", tag="psm")
                    for k in range(KTl):
                        nc.tensor.matmul(pad_ps[:, k, :], lhsT=selT[:, k, :],
                                         rhs=ald_w[:], start=True, stop=True)
                    # e = als[src] + ald[dst] ; ex = exp(lrelu(e))
                    e_t = sb.tile([128, KTl, 8], DT_F32, tag="e")
                    nc.vector.tensor_tensor(out=e_t[:], in0=hg[:, :, AO:AO + 8],
                                            in1=pad_ps[:], op=ALU.add)
                    lr = sb.tile([128, KTl, 8], DT_F32, tag="lr")
                    nc.scalar.activation(lr[:], e_t[:], AF.Lrelu, alpha=NEG_SLOPE)
                    ex = sb.tile([128, KTl, 8], DT_F32, tag="ex")
                    nc.scalar.activation(ex[:], lr[:], AF.Exp)
                    exb = sb.tile([128, KTl, 8], DT_BF, tag="exb")
                    nc.vector.tensor_copy(out=exb[:], in_=ex[:])
                    # denominator
                    pden = p_sm.tile([128, 8], DT_F32, space="PSUM", tag="psm")
                    for k in range(KTl):
                        nc.tensor.matmul(pden[:], lhsT=sel[:, k, :],
                                         rhs=exb[:, k, :],
                                         start=(k == 0), stop=(k == KT - 1))
                    den = sb.tile([128, 8], DT_F32, tag="den")
                    nc.vector.tensor_scalar_add(den[:], pden[:], 1e-16)
                    rec = sb.tile([128, 8], DT_F32, tag="rec")
                    nc.vector.reciprocal(rec[:], den[:])
                    # weighted messages + scatter-add matmuls
                    msg = sb.tile([128, KTl, FH], DT_BF, tag="msg")
                    hg4 = _bcast(hg[:], [hg[:].ap[0], [RWG, KTl], [CW, 8], [1, CW]])
                    ex4 = _bcast(exb[:], [exb[:].ap[0], [8, KTl], [1, 8], [0, CW]])
                    ms4 = _bcast(msg[:], [msg[:].ap[0], [FH, KTl], [CW, 8], [1, CW]])
                    nc.vector.tensor_tensor(out=ms4, in0=hg4, in1=ex4, op=ALU.mult)
                    pout = p_big.tile([128, FH], DT_F32, space="PSUM", tag="pbig")
                    for k in range(KTl):
                        nc.tensor.matmul(pout[:], lhsT=sel[:, k, :],
                                         rhs=msg[:, k, :],
                                         start=(k == 0), stop=(k == KT - 1))
                    # normalize
                    onrm = sb.tile([128, FH], DT_F32 if lay3 else DT_BF, tag="onrm")
                    po4 = _bcast(pout[:], [pout[:].ap[0], [CW, 8], [1, CW]])
                    rc4 = _bcast(rec[:], [rec[:].ap[0], [1, 8], [0, CW]])
                    on4 = _bcast(onrm[:], [onrm[:].ap[0], [CW, 8], [1, CW]])
                    nc.vector.tensor_tensor(out=on4, in0=po4, in1=rc4, op=ALU.mult)
                    if lay3:
                        hm = sb.tile([128, C3], DT_F32, tag="hm")
                        on_T = _bcast(onrm[:], [onrm[:].ap[0], [1, C3], [C3, 8]])
                        nc.vector.reduce_sum(hm[:], on_T, axis=mybir.AxisListType.X)
                        nc.vector.tensor_scalar_mul(hm[:], hm[:], 0.125)
                        nc.vector.tensor_add(out=hm[:], in0=hm[:], in1=t_b3[:])
                        mx = sb.tile([128, 1], DT_F32, tag="mx")
                        nc.vector.reduce_max(mx[:], hm[:], axis=mybir.AxisListType.X)
                        xc = sb.tile([128, C3], DT_F32, tag="xc")
                        nc.vector.tensor_tensor(out=xc[:], in0=hm[:],
                                                in1=mx[:].to_broadcast([128, C3]),
                                                op=ALU.subtract)
                        e5 = sb.tile([128, C3], DT_F32, tag="e5")
                        nc.scalar.activation(e5[:], xc[:], AF.Exp)
                        s5 = sb.tile([128, 1], DT_F32, tag="s5")
                        nc.vector.reduce_sum(s5[:], e5[:], axis=mybir.AxisListType.X)
                        lg = sb.tile([128, 1], DT_F32, tag="lg")
                        nc.scalar.activation(lg[:], s5[:], AF.Ln)
                        res = sb.tile([128, C3], DT_F32, tag="res")
                        nc.vector.tensor_tensor(out=res[:], in0=xc[:],
                                                in1=lg[:].to_broadcast([128, C3]),
                                                op=ALU.subtract)
                        rows = min(128, SHARD - w * 128)
                        nc.sync.dma_start(out=OUT[w * 128:w * 128 + rows, :],
                                          in_=res[:rows])
                        continue
                    # bias + relu -> x_next (bf16)
                    xb = sb.tile([128, F], DT_BF, tag="xb")
                    nc.vector.tensor_add(out=xb[:], in0=onrm[:],
                                         in1=t_b1[:] if layer == 1 else t_b2[:])
                    xn = sb.tile([128, F], DT_BF, tag="xn")
                    nc.scalar.activation(xn[:], xb[:], AF.Relu)
                    # transpose x_next, project to next layer
                    xt_ps = p_tr.tile([128, 8, 128], DT_BF, space="PSUM", tag="ptr")
                    for ch in range(4):
                        nc.tensor.transpose(xt_ps[:, ch, :],
                                            xn[:, ch * 128:(ch + 1) * 128],
                                            ident_bf[:])
                    xnT = sb.tile([128, 4, 128], DT_BF, tag="xnT")
                    nc.scalar.activation(xnT[:], xt_ps[:, 0:4, :], AF.Copy)
                    wN = w2 if layer == 1 else w3
                    wwaN = wwa2 if layer == 1 else wwa3
                    FN = F if layer == 1 else F3
                    ph = p_big.tile([128, FN], DT_F32, space="PSUM", tag="pbig")
                    pa = p_sm.tile([128, 16], DT_F32, space="PSUM", tag="psm")
                    for ch in range(4):
                        nc.tensor.matmul(ph[:], lhsT=xnT[:, ch, :],
                                         rhs=wN[:, ch, :],
                                         start=(ch == 0), stop=(ch == 3))
                        nc.tensor.matmul(pa[:], lhsT=xnT[:, ch, :],
                                         rhs=wwaN[:, ch, :],
                                         start=(ch == 0), stop=(ch == 3))
                    stage_write(ph, pa, FN, exch2 if layer == 1 else exch3, w)

            edge_phase(1, HF1, exch1)
            nc.gpsimd.collective_compute("AllGather", ALU.bypass, replica_groups=rg,
                                         ins=[exch2[0:SHARD, :].opt()],
                                         outs=[HF2[:].opt()])
            edge_phase(2, HF2, exch2)
            nc.gpsimd.collective_compute("AllGather", ALU.bypass, replica_groups=rg,
                                         ins=[exch3[0:SHARD, :].opt()],
                                         outs=[HF3[:].opt()])
            edge_phase(3, HF3, exch3)
            tk = sb.tile([128, 1], DT_F32, tag="tick")
            nc.sync.dma_start(out=tk[:], in_=P["tick"][:])
            nc.sync.dma_start(out=TOCK[:], in_=tk[:])

    nc.finalize()
    return nc


_CACHE = {}
_last_in_maps = None


def kernel(**inputs):
    x = np.asarray(inputs["x"], np.float32)
    edge_index = np.asarray(inputs["edge_index"], np.int32)
    ixl_t, ixh_t, drow, kl, kh, esrc3, drow3, kt3 = host_prep(edge_index)
    if (kl, kh, kt3) not in _CACHE:
        _CACHE[(kl, kh, kt3)] = build_program(kl, kh, kt3)
    nc = _CACHE[(kl, kh, kt3)]

    W1 = np.asarray(inputs["W1"], np.float32)
    W2 = np.asarray(inputs["W2"], np.float32)
    W3 = np.asarray(inputs["W3"], np.float32)
    WWa1 = W1 @ np.concatenate(
        [blockdiag(np.asarray(inputs["as1"])), blockdiag(np.asarray(inputs["ad1"]))], 1)
    WWa2 = W2 @ np.concatenate(
        [blockdiag(np.asarray(inputs["as2"])), blockdiag(np.asarray(inputs["ad2"]))], 1)
    WWa3 = W3 @ np.concatenate(
        [blockdiag(np.asarray(inputs["as3"])), blockdiag(np.asarray(inputs["ad3"]))], 1)
    com = {
        "w1": W1.astype(BF),
        "wwa1": WWa1.astype(BF),
        "w2c": chunk_rows(W2).astype(BF),
        "wwa2c": chunk_rows(WWa2).astype(BF),
        "w3c": chunk_rows(W3).astype(BF),
        "wwa3c": chunk_rows(WWa3).astype(BF),
        "b1t": np.tile(np.asarray(inputs["b1"], np.float32)[None, :], (128, 1)).astype(BF),
        "b2t": np.tile(np.asarray(inputs["b2"], np.float32)[None, :], (128, 1)).astype(BF),
        "b3t": np.tile(np.asarray(inputs["b3"], np.float32)[None, :], (128, 1)),
        "identt": np.eye(128, dtype=np.float32).astype(BF),
        "iotat": np.tile(np.arange(128, dtype=np.float32)[None, :], (128, 1)).astype(BF),
        "tick": np.zeros((128, 1), np.float32),
    }
    in_maps = []
    xTf = np.ascontiguousarray(x.T)
    for c in range(NC):
        m = dict(com)
        xc = np.zeros((12, NPAD), np.float32)
        lo = c * SHARD
        xc[:, :SHARD] = xTf[:, lo:lo + SHARD]
        m["xTc"] = xc
        m["ixl"] = ixl_t[c]
        m["ixh"] = ixh_t[c]
        m["drow"] = drow[c]
        m["esrc3"] = esrc3[c]
        m["drow3"] = drow3[c]
        in_maps.append(m)
    global _last_in_maps
    _last_in_maps = in_maps
    res = run_bass_kernel_spmd(nc, in_maps, list(range(NC)))
    return np.concatenate([res.results[c]["out"] for c in range(NC)], axis=0)
